# revision 29
# baseline (speedup 1.0000x reference)
"""Trainium2 Bass kernel for the GaussianModel occupancy-grid problem.

Strategy (v2: gaussian-major, matmul-reduce)
--------------------------------------------
occ[p] = sum_g w_g * exp(power(p, g)) with power a quadratic form in the
voxel coordinate p.  Per (gaussian, block) active pair ("slot"):

    power(p) = quad+lin(p_local) + const_g
    contribution = [w_g * e^{const_g}] * e^{quad+lin(p_local)}

The per-gaussian constant (and opacity weight) folds into a weight
w'_g = w_g e^{const_g}, so the device computes, per group of 128 slots:

    MM1 (PE):  power_ql[slot, vox] = coeff[18, 128slots]^T @ Phi[18, 64vox]
    ACT:       E = exp(power_ql)          (PSUM -> SBUF, fp32)
    MM2 (PE):  val[vox, piece] += E[slot, vox]^T @ w'[slot, piece]

Phi holds the 9 local-voxel features (x², y², z², xy, xz, yz, x, y, z)
scaled by 63 so every entry is a small odd-integer product — EXACT in
bf16.  Coefficients ship as bf16 hi + bf16 lo rows (rows 9..17 of Phi
duplicate rows 0..8), recovering fp32-level accuracy on the power while
keeping the 1-cycle/column bf16 matmul rate.  MM2 contracts over the
partition axis (slots), so the segmented gaussian reduction costs ~one
PE column per block piece and the vector engine is not on the critical
path at all.  Everything downstream of MM1 is fp32.

Blocks are dealt LPT-style across the 8 cores by active-gaussian count;
all cores run one SPMD program sized by the per-rank maxima.
"""

import numpy as np
import ml_dtypes

NB = 16          # num_blocks
RES = 64         # resolution
SPLIT = 4        # voxels per block side
N_CORES = 8
KF = 20          # 10 hi + 10 lo coefficient rows (incl. per-slot shift)
GRP = 128        # slots per group (MM2 contraction width)
VOX = 64         # voxels per block
TILE_G = 24      # groups per PSUM tile (24*64 = 1536 cols = 3 banks)

BF16 = ml_dtypes.bfloat16

_CACHE = {}


def _host_prep(_xyz, _scaling, _rotation, _opacity):
    """Mirror of the reference's per-gaussian preprocessing (numpy fp32)."""
    f32 = np.float32
    opac = (1.0 / (1.0 + np.exp(-_opacity[:, 0].astype(f32)))).astype(f32)
    keep = opac > 0.005
    opa = np.where(keep, opac, f32(0.0)).astype(f32)

    BIG = f32(1e10)
    mn = np.min(np.where(keep[:, None], _xyz, BIG), axis=0)
    mx = np.max(np.where(keep[:, None], _xyz, -BIG), axis=0)
    center = ((mn + mx) / 2).astype(f32)
    scale = (f32(1.8) / np.max(mx - mn)).astype(f32)
    xyzs = ((_xyz - center) * scale).astype(f32)
    stds = (np.exp(_scaling) * scale).astype(f32)

    q = (_rotation / np.linalg.norm(_rotation, axis=1, keepdims=True)).astype(f32)
    r, x, y, z = q[:, 0], q[:, 1], q[:, 2], q[:, 3]
    R = np.stack([
        np.stack([1 - 2 * (y * y + z * z), 2 * (x * y - r * z), 2 * (x * z + r * y)], -1),
        np.stack([2 * (x * y + r * z), 1 - 2 * (x * x + z * z), 2 * (y * z - r * x)], -1),
        np.stack([2 * (x * z - r * y), 2 * (y * z + r * x), 1 - 2 * (x * x + y * y)], -1),
    ], axis=1).astype(f32)
    L = R * stds[:, None, :]
    C = np.einsum('nij,nkj->nik', L, L).astype(f32)
    a, b, c = C[:, 0, 0], C[:, 0, 1], C[:, 0, 2]
    d, e, f = C[:, 1, 1], C[:, 1, 2], C[:, 2, 2]
    inv_det = (1.0 / (a * d * f + 2 * e * c * b - e * e * a - c * c * d
                      - b * b * f + 1e-24)).astype(f32)
    ia = ((d * f - e * e) * inv_det).astype(f32)
    ib = ((e * c - b * f) * inv_det).astype(f32)
    ic = ((e * b - c * d) * inv_det).astype(f32)
    id_ = ((a * f - c * c) * inv_det).astype(f32)
    ie = ((b * c - e * a) * inv_det).astype(f32)
    if_ = ((a * d - b * b) * inv_det).astype(f32)
    return xyzs, opa, (ia, ib, ic, id_, ie, if_)


def _build_workload(xyzs, opa):
    """Per-block active gaussian lists -> LPT core assignment -> group/piece
    packing and the shared SPMD schedule."""
    f32 = np.float32
    lin = np.linspace(-1.0, 1.0, RES).astype(f32)
    relax = f32((2.0 / NB) * 1.5)
    gx, gy, gz = xyzs[:, 0], xyzs[:, 1], xyzs[:, 2]
    act = opa > 0

    vmin = lin[np.arange(NB) * SPLIT] - relax
    vmax = lin[np.arange(NB) * SPLIT + SPLIT - 1] + relax
    Fx = (gx[None, :] > vmin[:, None]) & (gx[None, :] < vmax[:, None])
    Fy = (gy[None, :] > vmin[:, None]) & (gy[None, :] < vmax[:, None])
    Fz = ((gz[None, :] > vmin[:, None]) & (gz[None, :] < vmax[:, None])) & act

    blocks = []  # (n, bi, bj, bk, idx)
    for bi in range(NB):
        fx = Fx[bi]
        for bj in range(NB):
            fxy = fx & Fy[bj]
            if not fxy.any():
                continue
            for bk in range(NB):
                un = fxy & Fz[bk]
                idx = np.nonzero(un)[0]
                if idx.size:
                    blocks.append((idx.size, bi, bj, bk, idx))
    blocks.sort(key=lambda t: -t[0])

    # LPT deal by slot count
    loads = [0] * N_CORES
    core_blocks = [[] for _ in range(N_CORES)]
    for blk in blocks:
        c = min(range(N_CORES), key=lambda i: loads[i])
        core_blocks[c].append(blk)
        loads[c] += blk[0]

    # per-core group/piece packing
    per_core = []   # per core: list over groups of list of pieces
    g_counts = []
    for c in range(N_CORES):
        pieces_by_group = []
        cur = []       # pieces of current group
        fill = 0
        for (n, bi, bj, bk, idx) in core_blocks[c]:
            off = 0
            while off < n:
                take = min(n - off, GRP - fill)
                cur.append((bi, bj, bk, idx[off:off + take], fill))
                fill += take
                off += take
                if fill == GRP:
                    pieces_by_group.append(cur)
                    cur = []
                    fill = 0
        if cur:
            pieces_by_group.append(cur)
        per_core.append(pieces_by_group)
        g_counts.append(len(pieces_by_group))

    G = max(g_counts)
    nj = []
    for g in range(G):
        m = 0
        for c in range(N_CORES):
            if g < g_counts[c]:
                m = max(m, len(per_core[c][g]))
        nj.append(m)
    # a matmul output may not cross a PSUM bank (512 f32 cols): bump any
    # group whose column range would straddle a multiple of 512
    joff = np.zeros(G + 1, np.int64)
    j = 0
    for g in range(G):
        if (j // 512) != ((j + nj[g] - 1) // 512):
            j = (j // 512 + 1) * 512
        joff[g] = j
        j += nj[g]
    joff[G] = j
    J = int(j)
    assert J <= 1024, f"val columns {J} exceed 2 PSUM banks"

    # tile plan: a half-size warm-up tile so the first exp starts early,
    # then full 24-group tiles (two alternating 3-bank PSUM pools)
    tiles = []
    g0 = 0
    first = TILE_G // 2
    if G > first:
        tiles.append((0, first))
        g0 = first
    while g0 < G:
        g1 = min(G, g0 + TILE_G)
        tiles.append((g0, g1))
        g0 = g1


    schedule = {"G": G, "nj": tuple(nj), "joff": joff, "J": J,
                "tiles": tuple(tiles)}
    return schedule, per_core


def _build_inputs(schedule, per_core, xyzs, opa, inv):
    """Build phi/coeff/wind arrays per core + host assembly map."""
    f32 = np.float32
    ia, ib, ic, id_, ie, if_ = inv
    lin = np.linspace(-1.0, 1.0, RES).astype(f32)
    gx, gy, gz = xyzs[:, 0], xyzs[:, 1], xyzs[:, 2]
    G, joff, J = schedule["G"], schedule["joff"], schedule["J"]

    # Phi: 64 voxels of one block, p = ix*16 + iy*4 + iz; scaled coords
    # 63*(local offset) = odd ints {-3,-1,1,3} -> all features exact in bf16
    k = np.array([-3.0, -1.0, 1.0, 3.0], f32)
    X, Y, Z = np.meshgrid(k, k, k, indexing='ij')
    X, Y, Z = X.ravel(), Y.ravel(), Z.ravel()
    ones = np.ones(VOX, f32)
    feats = np.stack([X * X, Y * Y, Z * Z, X * Y, X * Z, Y * Z,
                      X, Y, Z, ones], 0)
    phi = np.zeros((KF, VOX), f32)
    phi[0:10] = feats
    phi[10:20] = feats
    phi_b = phi.astype(BF16)   # exact: small ints

    # local scale: voxel pitch is 2/63, offsets are k/2 * pitch = k/63
    s = f32(1.0) / f32(63.0)
    s2 = s * s

    logopa = np.where(opa > 0, np.log(np.maximum(opa, 1e-30)), f32(-1e10))

    in_maps = []
    assembly = []   # (core, col_j, bi, bj, bk)
    for c in range(N_CORES):
        coeff = np.zeros((10, G * GRP), f32)
        wind = np.zeros((GRP, J), f32)
        for g, pieces in enumerate(per_core[c]):
            for pi, (bi, bj, bk, idx, slot0) in enumerate(pieces):
                cx = f32((lin[bi * 4] + lin[bi * 4 + 3]) * 0.5)
                cy = f32((lin[bj * 4] + lin[bj * 4 + 3]) * 0.5)
                cz = f32((lin[bk * 4] + lin[bk * 4 + 3]) * 0.5)
                g0x = (gx[idx] - cx).astype(f32)
                g0y = (gy[idx] - cy).astype(f32)
                g0z = (gz[idx] - cz).astype(f32)
                A_ = ia[idx]; B_ = ib[idx]; Cc = ic[idx]
                D_ = id_[idx]; E_ = ie[idx]; F_ = if_[idx]
                Agx = A_ * g0x + B_ * g0y + Cc * g0z
                Agy = B_ * g0x + D_ * g0y + E_ * g0z
                Agz = Cc * g0x + E_ * g0y + F_ * g0z
                const = (-0.5 * (g0x * Agx + g0y * Agy + g0z * Agz)).astype(f32)

                o = g * GRP + slot0
                n = idx.size
                coeff[0, o:o + n] = -0.5 * A_ * s2
                coeff[1, o:o + n] = -0.5 * D_ * s2
                coeff[2, o:o + n] = -0.5 * F_ * s2
                coeff[3, o:o + n] = -B_ * s2
                coeff[4, o:o + n] = -Cc * s2
                coeff[5, o:o + n] = -E_ * s2
                coeff[6, o:o + n] = Agx * s
                coeff[7, o:o + n] = Agy * s
                coeff[8, o:o + n] = Agz * s
                # full constant rides the ones-feature row (hi/lo split), so
                # the exp input is the true power <= 0 — the HW exp table is
                # only trustworthy on non-positive inputs
                coeff[9, o:o + n] = const
                wind[slot0:slot0 + n, int(joff[g]) + pi] = opa[idx]
                assembly.append((c, int(joff[g]) + pi, bi, bj, bk))
        hi = coeff.astype(BF16)
        lo = (coeff - hi.astype(f32)).astype(BF16)
        cfull = np.concatenate([hi, lo], axis=0)  # [20, G*128] bf16
        # phi lives in the first 64 columns of the coeff tensor (one DMA)
        merged = np.concatenate([phi_b, cfull], axis=1)  # [20, 64+G*128]
        in_maps.append({"coeff": merged, "wind": wind.astype(f32)})
    return in_maps, assembly


def _build_program(schedule):
    import concourse.bass as bass  # noqa: F401
    import concourse.bacc as bacc
    import concourse.tile as tile
    import concourse.mybir as mybir
    from concourse.tile_rust import add_dep_helper

    G = schedule["G"]
    joff = schedule["joff"]
    J = schedule["J"]
    tiles = schedule["tiles"]
    T = len(tiles)
    f32 = mybir.dt.float32
    bf16 = mybir.dt.bfloat16

    nc = bacc.Bacc("TRN2", target_bir_lowering=False, debug=False,
                   num_devices=N_CORES)
    ccols = VOX + G * GRP   # phi occupies the first 64 columns
    coeff_d = nc.dram_tensor("coeff", [KF, ccols], bf16, kind="ExternalInput")
    wind_d = nc.dram_tensor("wind", [GRP, J], f32, kind="ExternalInput")
    val_d = nc.dram_tensor("val", [VOX, J], f32, kind="ExternalOutput")

    tile_cols = TILE_G * VOX

    with tile.TileContext(nc) as tc:
        with tc.tile_pool(name="inp", bufs=1) as inp, \
             tc.tile_pool(name="es", bufs=5) as es, \
             tc.tile_pool(name="vs", bufs=1) as vs, \
             tc.tile_pool(name="psa", bufs=1, space="PSUM") as psa, \
             tc.tile_pool(name="psb", bufs=1, space="PSUM") as psb, \
             tc.tile_pool(name="vp", bufs=1, space="PSUM") as vp:
            coeff_t = inp.tile([KF, ccols], bf16, name="coeff_sb")
            wind_t = inp.tile([GRP, J], f32, name="wind_sb")
            val_sb = vs.tile([VOX, J], f32, name="val_sb")
            val_ps = vp.tile([VOX, J], f32, name="val_ps", tag="vp")
            phi_t = coeff_t[:, 0:VOX]
            # padding columns (per-core nj slack + bank-boundary bumps) are
            # never written by MM2s but are read by the drain copies
            nc.vector.memset(val_ps, 0.0)

            # coeff chunks (SP queue): tiles 0+1 merged so the warm-up isn't
            # gated on a second DGE pass; wind slots in after the third
            # chunk — its consumers (MM2s) trail the MM1 stream by 2 tiles
            merge01 = T >= 2
            wind_after = min(3, T - 1)
            for t, (g0, g1) in enumerate(tiles):
                if t == 1 and merge01:
                    continue
                c0 = VOX + g0 * GRP if t > 0 else 0
                c1 = VOX + (tiles[1][1] if (t == 0 and merge01) else g1) * GRP
                nc.sync.dma_start(out=coeff_t[:, c0:c1],
                                  in_=coeff_d.ap()[:, c0:c1])
                if t == wind_after:
                    nc.sync.dma_start(out=wind_t, in_=wind_d.ap())

            ets = {}
            for t, (g0, g1) in enumerate(tiles):
                used = (g1 - g0) * VOX
                pool = psa if t % 2 == 0 else psb
                pt = pool.tile([128, tile_cols], f32, name=f"pt{t}",
                               tag=f"pt{t % 2}")
                last_mm1 = None
                for k, g in enumerate(range(g0, g1)):
                    o = VOX + g * GRP
                    last_mm1 = nc.tensor.matmul(
                        pt[:, k * VOX:(k + 1) * VOX],
                        coeff_t[:, o:o + GRP],
                        phi_t,
                        start=True, stop=True)
                et = es.tile([128, tile_cols], f32, name=f"et{t}", tag="et")
                ets[t] = et
                nc.scalar.activation(et[:, :used], pt[:, :used],
                                     mybir.ActivationFunctionType.Exp)
                # software pipelining: tile t-2's MM2s are forced AFTER this
                # tile's MM1s (the scheduler reorders freely otherwise) so an
                # in-flight exp never has MM2s parked with unsatisfied waits
                # ahead of later MM1s on the in-order PE
                if t >= 2:
                    _emit_mm2s(nc, schedule, t - 2, ets[t - 2], wind_t,
                               val_ps, add_dep_helper, last_mm1)
                    del ets[t - 2]
            _emit_mm2s(nc, schedule, T - 2, ets[T - 2], wind_t, val_ps,
                       None, None)
            # val columns for tiles <= T-2 are final: drain them while the
            # last tile's exp + MM2s run.  The final chunk is padded to 128
            # cols so its DMA descriptors are >= 512B (avoids the 2x
            # small-descriptor penalty) and copied on the idle ACT engine.
            jcut = min(int(joff[tiles[T - 1][0]]) // 4 * 4, max(0, J - 128))
            if jcut:
                nc.vector.tensor_copy(out=val_sb[:, :jcut],
                                      in_=val_ps[:, :jcut])
                nc.sync.dma_start(out=val_d.ap()[:, :jcut],
                                  in_=val_sb[:, :jcut])
            _emit_mm2s(nc, schedule, T - 1, ets[T - 1], wind_t, val_ps,
                       None, None)
            nc.scalar.copy(val_sb[:, jcut:], val_ps[:, jcut:])
            nc.sync.dma_start(out=val_d.ap()[:, jcut:], in_=val_sb[:, jcut:])

    nc.compile()
    return nc


def _emit_mm2s(nc, schedule, t, et, wind_t, val_ps, add_dep, after_inst):
    nj = schedule["nj"]
    joff = schedule["joff"]
    g0, g1 = schedule["tiles"][t]
    for k, g in enumerate(range(g0, g1)):
        if nj[g] == 0:
            continue
        j0, j1 = int(joff[g]), int(joff[g]) + nj[g]
        mm2 = nc.tensor.matmul(
            val_ps[:, j0:j1],
            et[:, k * VOX:(k + 1) * VOX],
            wind_t[:, j0:j1],
            start=True, stop=True)
        if add_dep is not None and after_inst is not None:
            add_dep(mm2.ins, after_inst.ins,
                    reason="defer MM2 behind MM1s two tiles ahead")


def _assemble(schedule, assembly, results):
    occ = np.zeros((RES, RES, RES), np.float32)
    for (c, j, bi, bj, bk) in assembly:
        v = results[c]["val"][:, j].astype(np.float32).reshape(4, 4, 4)
        occ[bi * 4:bi * 4 + 4, bj * 4:bj * 4 + 4, bk * 4:bk * 4 + 4] += v
    return occ


def kernel(_xyz, _scaling, _rotation, _opacity, resolution, num_blocks):
    assert int(resolution) == RES and int(num_blocks) == NB, \
        f"kernel hardcoded for resolution=64 num_blocks=16, got {resolution}/{num_blocks}"
    try:
        import concourse.bass_utils as bass_utils  # noqa: F401
    except ImportError:
        import sys
        sys.path.insert(0, "/opt/trn_rl_repo")
        import concourse.bass_utils as bass_utils

    _xyz = np.asarray(_xyz, np.float32)
    _scaling = np.asarray(_scaling, np.float32)
    _rotation = np.asarray(_rotation, np.float32)
    _opacity = np.asarray(_opacity, np.float32)

    xyzs, opa, inv = _host_prep(_xyz, _scaling, _rotation, _opacity)
    schedule, per_core = _build_workload(xyzs, opa)
    in_maps, assembly = _build_inputs(schedule, per_core, xyzs, opa, inv)

    key = (schedule["G"], schedule["nj"])
    if key not in _CACHE:
        _CACHE.clear()
        _CACHE[key] = _build_program(schedule)
    nc = _CACHE[key]

    # the axon tunnel occasionally reports a transient
    # NRT_EXEC_UNIT_UNRECOVERABLE; it clears on retry
    import time
    last_err = None
    for attempt in range(4):
        try:
            res = bass_utils.run_bass_kernel_spmd(
                nc, in_maps, core_ids=list(range(N_CORES)))
            return _assemble(schedule, assembly, res.results)
        except Exception as e:  # noqa: BLE001
            last_err = e
            if "UNRECOVERABLE" not in str(e) and "UNAVAILABLE" not in str(e):
                raise
            time.sleep(10 * (attempt + 1))
    raise last_err


# revision 30
# speedup vs baseline: 1.0029x; 1.0029x over previous
"""Trainium2 Bass kernel for the GaussianModel occupancy-grid problem.

Strategy (v2: gaussian-major, matmul-reduce)
--------------------------------------------
occ[p] = sum_g w_g * exp(power(p, g)) with power a quadratic form in the
voxel coordinate p.  Per (gaussian, block) active pair ("slot"):

    power(p) = quad+lin(p_local) + const_g
    contribution = [w_g * e^{const_g}] * e^{quad+lin(p_local)}

The per-gaussian constant (and opacity weight) folds into a weight
w'_g = w_g e^{const_g}, so the device computes, per group of 128 slots:

    MM1 (PE):  power_ql[slot, vox] = coeff[18, 128slots]^T @ Phi[18, 64vox]
    ACT:       E = exp(power_ql)          (PSUM -> SBUF, fp32)
    MM2 (PE):  val[vox, piece] += E[slot, vox]^T @ w'[slot, piece]

Phi holds the 9 local-voxel features (x², y², z², xy, xz, yz, x, y, z)
scaled by 63 so every entry is a small odd-integer product — EXACT in
bf16.  Coefficients ship as bf16 hi + bf16 lo rows (rows 9..17 of Phi
duplicate rows 0..8), recovering fp32-level accuracy on the power while
keeping the 1-cycle/column bf16 matmul rate.  MM2 contracts over the
partition axis (slots), so the segmented gaussian reduction costs ~one
PE column per block piece and the vector engine is not on the critical
path at all.  Everything downstream of MM1 is fp32.

Blocks are dealt LPT-style across the 8 cores by active-gaussian count;
all cores run one SPMD program sized by the per-rank maxima.
"""

import numpy as np
import ml_dtypes

NB = 16          # num_blocks
RES = 64         # resolution
SPLIT = 4        # voxels per block side
N_CORES = 8
KF = 20          # 10 hi + 10 lo coefficient rows (incl. per-slot shift)
GRP = 128        # slots per group (MM2 contraction width)
VOX = 64         # voxels per block
TILE_G = 24      # groups per PSUM tile (24*64 = 1536 cols = 3 banks)

BF16 = ml_dtypes.bfloat16

_CACHE = {}


def _host_prep(_xyz, _scaling, _rotation, _opacity):
    """Mirror of the reference's per-gaussian preprocessing (numpy fp32)."""
    f32 = np.float32
    opac = (1.0 / (1.0 + np.exp(-_opacity[:, 0].astype(f32)))).astype(f32)
    keep = opac > 0.005
    opa = np.where(keep, opac, f32(0.0)).astype(f32)

    BIG = f32(1e10)
    mn = np.min(np.where(keep[:, None], _xyz, BIG), axis=0)
    mx = np.max(np.where(keep[:, None], _xyz, -BIG), axis=0)
    center = ((mn + mx) / 2).astype(f32)
    scale = (f32(1.8) / np.max(mx - mn)).astype(f32)
    xyzs = ((_xyz - center) * scale).astype(f32)
    stds = (np.exp(_scaling) * scale).astype(f32)

    q = (_rotation / np.linalg.norm(_rotation, axis=1, keepdims=True)).astype(f32)
    r, x, y, z = q[:, 0], q[:, 1], q[:, 2], q[:, 3]
    R = np.stack([
        np.stack([1 - 2 * (y * y + z * z), 2 * (x * y - r * z), 2 * (x * z + r * y)], -1),
        np.stack([2 * (x * y + r * z), 1 - 2 * (x * x + z * z), 2 * (y * z - r * x)], -1),
        np.stack([2 * (x * z - r * y), 2 * (y * z + r * x), 1 - 2 * (x * x + y * y)], -1),
    ], axis=1).astype(f32)
    L = R * stds[:, None, :]
    C = np.einsum('nij,nkj->nik', L, L).astype(f32)
    a, b, c = C[:, 0, 0], C[:, 0, 1], C[:, 0, 2]
    d, e, f = C[:, 1, 1], C[:, 1, 2], C[:, 2, 2]
    inv_det = (1.0 / (a * d * f + 2 * e * c * b - e * e * a - c * c * d
                      - b * b * f + 1e-24)).astype(f32)
    ia = ((d * f - e * e) * inv_det).astype(f32)
    ib = ((e * c - b * f) * inv_det).astype(f32)
    ic = ((e * b - c * d) * inv_det).astype(f32)
    id_ = ((a * f - c * c) * inv_det).astype(f32)
    ie = ((b * c - e * a) * inv_det).astype(f32)
    if_ = ((a * d - b * b) * inv_det).astype(f32)
    return xyzs, opa, (ia, ib, ic, id_, ie, if_)


def _build_workload(xyzs, opa):
    """Per-block active gaussian lists -> LPT core assignment -> group/piece
    packing and the shared SPMD schedule."""
    f32 = np.float32
    lin = np.linspace(-1.0, 1.0, RES).astype(f32)
    relax = f32((2.0 / NB) * 1.5)
    gx, gy, gz = xyzs[:, 0], xyzs[:, 1], xyzs[:, 2]
    act = opa > 0

    vmin = lin[np.arange(NB) * SPLIT] - relax
    vmax = lin[np.arange(NB) * SPLIT + SPLIT - 1] + relax
    Fx = (gx[None, :] > vmin[:, None]) & (gx[None, :] < vmax[:, None])
    Fy = (gy[None, :] > vmin[:, None]) & (gy[None, :] < vmax[:, None])
    Fz = ((gz[None, :] > vmin[:, None]) & (gz[None, :] < vmax[:, None])) & act

    blocks = []  # (n, bi, bj, bk, idx)
    for bi in range(NB):
        fx = Fx[bi]
        for bj in range(NB):
            fxy = fx & Fy[bj]
            if not fxy.any():
                continue
            for bk in range(NB):
                un = fxy & Fz[bk]
                idx = np.nonzero(un)[0]
                if idx.size:
                    blocks.append((idx.size, bi, bj, bk, idx))
    blocks.sort(key=lambda t: -t[0])

    # LPT deal by slot count
    loads = [0] * N_CORES
    core_blocks = [[] for _ in range(N_CORES)]
    for blk in blocks:
        c = min(range(N_CORES), key=lambda i: loads[i])
        core_blocks[c].append(blk)
        loads[c] += blk[0]

    # per-core group/piece packing
    per_core = []   # per core: list over groups of list of pieces
    g_counts = []
    for c in range(N_CORES):
        pieces_by_group = []
        cur = []       # pieces of current group
        fill = 0
        for (n, bi, bj, bk, idx) in core_blocks[c]:
            off = 0
            while off < n:
                take = min(n - off, GRP - fill)
                cur.append((bi, bj, bk, idx[off:off + take], fill))
                fill += take
                off += take
                if fill == GRP:
                    pieces_by_group.append(cur)
                    cur = []
                    fill = 0
        if cur:
            pieces_by_group.append(cur)
        per_core.append(pieces_by_group)
        g_counts.append(len(pieces_by_group))

    G = max(g_counts)
    nj = []
    for g in range(G):
        m = 0
        for c in range(N_CORES):
            if g < g_counts[c]:
                m = max(m, len(per_core[c][g]))
        nj.append(m)
    # a matmul output may not cross a PSUM bank (512 f32 cols): bump any
    # group whose column range would straddle a multiple of 512
    joff = np.zeros(G + 1, np.int64)
    j = 0
    for g in range(G):
        if (j // 512) != ((j + nj[g] - 1) // 512):
            j = (j // 512 + 1) * 512
        joff[g] = j
        j += nj[g]
    joff[G] = j
    J = int(j)
    assert J <= 1024, f"val columns {J} exceed 2 PSUM banks"

    # tile plan: a half-size warm-up tile so the first exp starts early,
    # then full 24-group tiles (two alternating 3-bank PSUM pools)
    tiles = []
    g0 = 0
    first = TILE_G // 2
    if G > first:
        tiles.append((0, first))
        g0 = first
    while g0 < G:
        g1 = min(G, g0 + TILE_G)
        tiles.append((g0, g1))
        g0 = g1


    schedule = {"G": G, "nj": tuple(nj), "joff": joff, "J": J,
                "tiles": tuple(tiles)}
    return schedule, per_core


def _build_inputs(schedule, per_core, xyzs, opa, inv):
    """Build phi/coeff/wind arrays per core + host assembly map."""
    f32 = np.float32
    ia, ib, ic, id_, ie, if_ = inv
    lin = np.linspace(-1.0, 1.0, RES).astype(f32)
    gx, gy, gz = xyzs[:, 0], xyzs[:, 1], xyzs[:, 2]
    G, joff, J = schedule["G"], schedule["joff"], schedule["J"]

    # Phi: 64 voxels of one block, p = ix*16 + iy*4 + iz; scaled coords
    # 63*(local offset) = odd ints {-3,-1,1,3} -> all features exact in bf16
    k = np.array([-3.0, -1.0, 1.0, 3.0], f32)
    X, Y, Z = np.meshgrid(k, k, k, indexing='ij')
    X, Y, Z = X.ravel(), Y.ravel(), Z.ravel()
    ones = np.ones(VOX, f32)
    feats = np.stack([X * X, Y * Y, Z * Z, X * Y, X * Z, Y * Z,
                      X, Y, Z, ones], 0)
    phi = np.zeros((KF, VOX), f32)
    phi[0:10] = feats
    phi[10:20] = feats
    phi_b = phi.astype(BF16)   # exact: small ints

    # local scale: voxel pitch is 2/63, offsets are k/2 * pitch = k/63
    s = f32(1.0) / f32(63.0)
    s2 = s * s

    logopa = np.where(opa > 0, np.log(np.maximum(opa, 1e-30)), f32(-1e10))

    in_maps = []
    assembly = []   # (core, col_j, bi, bj, bk)
    for c in range(N_CORES):
        coeff = np.zeros((10, G * GRP), f32)
        wind = np.zeros((GRP, J), f32)
        for g, pieces in enumerate(per_core[c]):
            for pi, (bi, bj, bk, idx, slot0) in enumerate(pieces):
                cx = f32((lin[bi * 4] + lin[bi * 4 + 3]) * 0.5)
                cy = f32((lin[bj * 4] + lin[bj * 4 + 3]) * 0.5)
                cz = f32((lin[bk * 4] + lin[bk * 4 + 3]) * 0.5)
                g0x = (gx[idx] - cx).astype(f32)
                g0y = (gy[idx] - cy).astype(f32)
                g0z = (gz[idx] - cz).astype(f32)
                A_ = ia[idx]; B_ = ib[idx]; Cc = ic[idx]
                D_ = id_[idx]; E_ = ie[idx]; F_ = if_[idx]
                Agx = A_ * g0x + B_ * g0y + Cc * g0z
                Agy = B_ * g0x + D_ * g0y + E_ * g0z
                Agz = Cc * g0x + E_ * g0y + F_ * g0z
                const = (-0.5 * (g0x * Agx + g0y * Agy + g0z * Agz)).astype(f32)

                o = g * GRP + slot0
                n = idx.size
                coeff[0, o:o + n] = -0.5 * A_ * s2
                coeff[1, o:o + n] = -0.5 * D_ * s2
                coeff[2, o:o + n] = -0.5 * F_ * s2
                coeff[3, o:o + n] = -B_ * s2
                coeff[4, o:o + n] = -Cc * s2
                coeff[5, o:o + n] = -E_ * s2
                coeff[6, o:o + n] = Agx * s
                coeff[7, o:o + n] = Agy * s
                coeff[8, o:o + n] = Agz * s
                # full constant rides the ones-feature row (hi/lo split), so
                # the exp input is the true power <= 0 — the HW exp table is
                # only trustworthy on non-positive inputs
                coeff[9, o:o + n] = const
                wind[slot0:slot0 + n, int(joff[g]) + pi] = opa[idx]
                assembly.append((c, int(joff[g]) + pi, bi, bj, bk))
        hi = coeff.astype(BF16)
        lo = (coeff - hi.astype(f32)).astype(BF16)
        cfull = np.concatenate([hi, lo], axis=0)  # [20, G*128] bf16
        # phi lives in the first 64 columns of the coeff tensor (one DMA)
        merged = np.concatenate([phi_b, cfull], axis=1)  # [20, 64+G*128]
        in_maps.append({"coeff": merged, "wind": wind.astype(f32)})
    return in_maps, assembly


def _build_program(schedule):
    import concourse.bass as bass  # noqa: F401
    import concourse.bacc as bacc
    import concourse.tile as tile
    import concourse.mybir as mybir
    from concourse.tile_rust import add_dep_helper

    G = schedule["G"]
    joff = schedule["joff"]
    J = schedule["J"]
    tiles = schedule["tiles"]
    T = len(tiles)
    f32 = mybir.dt.float32
    bf16 = mybir.dt.bfloat16

    nc = bacc.Bacc("TRN2", target_bir_lowering=False, debug=False,
                   num_devices=N_CORES)
    ccols = VOX + G * GRP   # phi occupies the first 64 columns
    coeff_d = nc.dram_tensor("coeff", [KF, ccols], bf16, kind="ExternalInput")
    wind_d = nc.dram_tensor("wind", [GRP, J], f32, kind="ExternalInput")
    val_d = nc.dram_tensor("val", [VOX, J], f32, kind="ExternalOutput")

    tile_cols = TILE_G * VOX

    with tile.TileContext(nc) as tc:
        with tc.tile_pool(name="inp", bufs=1) as inp, \
             tc.tile_pool(name="es", bufs=5) as es, \
             tc.tile_pool(name="vs", bufs=1) as vs, \
             tc.tile_pool(name="psa", bufs=1, space="PSUM") as psa, \
             tc.tile_pool(name="psb", bufs=1, space="PSUM") as psb, \
             tc.tile_pool(name="vp", bufs=1, space="PSUM") as vp:
            coeff_t = inp.tile([KF, ccols], bf16, name="coeff_sb")
            wind_t = inp.tile([GRP, J], f32, name="wind_sb")
            val_sb = vs.tile([VOX, J], f32, name="val_sb")
            val_ps = vp.tile([VOX, J], f32, name="val_ps", tag="vp")
            phi_t = coeff_t[:, 0:VOX]
            # padding columns (per-core nj slack + bank-boundary bumps) are
            # never written by MM2s but are read by the drain copies
            nc.vector.memset(val_ps, 0.0)

            # coeff chunks (SP queue): tiles 0+1 merged so the warm-up isn't
            # gated on a second DGE pass; wind slots in after the third
            # chunk — its consumers (MM2s) trail the MM1 stream by 2 tiles
            merge01 = T >= 2
            wind_after = min(3, T - 1)
            for t, (g0, g1) in enumerate(tiles):
                if t == 1 and merge01:
                    continue
                c0 = VOX + g0 * GRP if t > 0 else 0
                c1 = VOX + (tiles[1][1] if (t == 0 and merge01) else g1) * GRP
                nc.sync.dma_start(out=coeff_t[:, c0:c1],
                                  in_=coeff_d.ap()[:, c0:c1])
                if t == wind_after:
                    nc.sync.dma_start(out=wind_t, in_=wind_d.ap())

            ets = {}
            for t, (g0, g1) in enumerate(tiles):
                used = (g1 - g0) * VOX
                pool = psa if t % 2 == 0 else psb
                pt = pool.tile([128, tile_cols], f32, name=f"pt{t}",
                               tag=f"pt{t % 2}")
                last_mm1 = None
                for k, g in enumerate(range(g0, g1)):
                    o = VOX + g * GRP
                    last_mm1 = nc.tensor.matmul(
                        pt[:, k * VOX:(k + 1) * VOX],
                        coeff_t[:, o:o + GRP],
                        phi_t,
                        start=True, stop=True)
                et = es.tile([128, tile_cols], f32, name=f"et{t}", tag="et")
                ets[t] = et
                nc.scalar.activation(et[:, :used], pt[:, :used],
                                     mybir.ActivationFunctionType.Exp)
                # software pipelining: tile t-2's MM2s are forced AFTER this
                # tile's MM1s (the scheduler reorders freely otherwise) so an
                # in-flight exp never has MM2s parked with unsatisfied waits
                # ahead of later MM1s on the in-order PE
                if t >= 2:
                    _emit_mm2s(nc, schedule, t - 2, ets[t - 2], wind_t,
                               val_ps, add_dep_helper, last_mm1)
                    del ets[t - 2]
            _emit_mm2s(nc, schedule, T - 2, ets[T - 2], wind_t, val_ps,
                       None, None)
            # val columns for tiles <= T-2 are final: drain them while the
            # last tile's exp + MM2s run.  The final chunk is padded to 128
            # cols so its DMA descriptors are >= 512B (avoids the 2x
            # small-descriptor penalty) and copied on the idle ACT engine.
            jcut = min(int(joff[tiles[T - 1][0]]) // 4 * 4, max(0, J - 128))
            if jcut:
                nc.vector.tensor_copy(out=val_sb[:, :jcut],
                                      in_=val_ps[:, :jcut])
                nc.sync.dma_start(out=val_d.ap()[:, :jcut],
                                  in_=val_sb[:, :jcut])
            _emit_mm2s(nc, schedule, T - 1, ets[T - 1], wind_t, val_ps,
                       None, None)
            nc.vector.tensor_copy(out=val_sb[:, jcut:], in_=val_ps[:, jcut:])
            nc.sync.dma_start(out=val_d.ap()[:, jcut:], in_=val_sb[:, jcut:])

    nc.compile()
    return nc


def _emit_mm2s(nc, schedule, t, et, wind_t, val_ps, add_dep, after_inst):
    nj = schedule["nj"]
    joff = schedule["joff"]
    g0, g1 = schedule["tiles"][t]
    for k, g in enumerate(range(g0, g1)):
        if nj[g] == 0:
            continue
        j0, j1 = int(joff[g]), int(joff[g]) + nj[g]
        mm2 = nc.tensor.matmul(
            val_ps[:, j0:j1],
            et[:, k * VOX:(k + 1) * VOX],
            wind_t[:, j0:j1],
            start=True, stop=True)
        if add_dep is not None and after_inst is not None:
            add_dep(mm2.ins, after_inst.ins,
                    reason="defer MM2 behind MM1s two tiles ahead")


def _assemble(schedule, assembly, results):
    occ = np.zeros((RES, RES, RES), np.float32)
    for (c, j, bi, bj, bk) in assembly:
        v = results[c]["val"][:, j].astype(np.float32).reshape(4, 4, 4)
        occ[bi * 4:bi * 4 + 4, bj * 4:bj * 4 + 4, bk * 4:bk * 4 + 4] += v
    return occ


def kernel(_xyz, _scaling, _rotation, _opacity, resolution, num_blocks):
    assert int(resolution) == RES and int(num_blocks) == NB, \
        f"kernel hardcoded for resolution=64 num_blocks=16, got {resolution}/{num_blocks}"
    try:
        import concourse.bass_utils as bass_utils  # noqa: F401
    except ImportError:
        import sys
        sys.path.insert(0, "/opt/trn_rl_repo")
        import concourse.bass_utils as bass_utils

    _xyz = np.asarray(_xyz, np.float32)
    _scaling = np.asarray(_scaling, np.float32)
    _rotation = np.asarray(_rotation, np.float32)
    _opacity = np.asarray(_opacity, np.float32)

    xyzs, opa, inv = _host_prep(_xyz, _scaling, _rotation, _opacity)
    schedule, per_core = _build_workload(xyzs, opa)
    in_maps, assembly = _build_inputs(schedule, per_core, xyzs, opa, inv)

    key = (schedule["G"], schedule["nj"])
    if key not in _CACHE:
        _CACHE.clear()
        _CACHE[key] = _build_program(schedule)
    nc = _CACHE[key]

    # the axon tunnel occasionally reports a transient
    # NRT_EXEC_UNIT_UNRECOVERABLE; it clears on retry
    import time
    last_err = None
    for attempt in range(4):
        try:
            res = bass_utils.run_bass_kernel_spmd(
                nc, in_maps, core_ids=list(range(N_CORES)))
            return _assemble(schedule, assembly, res.results)
        except Exception as e:  # noqa: BLE001
            last_err = e
            if "UNRECOVERABLE" not in str(e) and "UNAVAILABLE" not in str(e):
                raise
            time.sleep(10 * (attempt + 1))
    raise last_err
